# revision 13
# baseline (speedup 1.0000x reference)
"""Weighted-BCE loss kernel for Trainium2 (8 NeuronCores, SPMD data-parallel).

Reference math (torch-style BCELoss with class-balancing weights):
    n   = len(x), s = sum(gt)
    w0  = n / (2*(n-s)),  w1 = n / (2*s)
    L1  = max(log(x),     -100)
    L0  = max(log1p(-x),  -100)
    loss = mean( where(gt==0, w0, w1) * -(gt*L1 + (1-gt)*L0) )

The weights depend only on the GLOBAL positive count s, so the loss
decomposes into 4 global sums computed shard-locally:
    A = sum(gt * L1),  B = sum(gt * L0),  C = sum(L0),  s = sum(gt)
    loss = -( A/(2s) + (C-B)/(2(n-s)) )

Each core processes a 1/8 shard laid out [128 partitions, 16384 free].
Structure (v8):
  - DMA and compute granularity are decoupled: inputs stream as a few
    LARGE transfers (x_i, gt_i interleaved on the single SP HWDGE ring)
    to amortize the ~1.1us per-transfer trigger/DGE dead time observed
    between back-to-back transfers, while compute runs in 2048-column
    chunks sliced from the landed tiles.
  - Per 2048-chunk, both engines carry ~5.4us so neither starves:
      ACT: Ln(x); Ln(1-x) (affine scale=-1,bias=1, accum_out -> C);
           Copy+accum over the first half of gt's columns (-> S_act).
      DVE: S-STT over gt's other half FIRST (needs only gt, fills the
           wait on lnx), then A and B clamp+mult+accum STTs.
    All f32 (bf16/int32 operand mixing measured slower on DVE).
  - Small first DMA tile starts compute early; small last tiles
    shorten the tail.
Host gathers the partial-sum columns (A|B|C|Sa|Sd groups) from all 8
cores and finishes the tiny all-reduce + final scalar math in float64.
"""

import numpy as np
from contextlib import ExitStack

import concourse.bass as bass
import concourse.bacc as bacc
import concourse.mybir as mybir
import concourse.tile as tile
from concourse.alu_op_type import AluOpType
from concourse.bass_utils import run_bass_kernel_spmd

N_TOTAL = 16777216
N_CORES = 8
PER_CORE = N_TOTAL // N_CORES   # 2097152
P = 128
FD = PER_CORE // P              # 16384 free elements per partition
DMA_TILES = [1024, 2048, 4096, 4096, 4096, 1024]
assert sum(DMA_TILES) == FD
CHUNK = 2048                    # compute granularity within a DMA tile
# compute chunks: (dma_tile_idx, start_col_within_tile, width)
CHUNKS = []
for _ti, _w in enumerate(DMA_TILES):
    _o = 0
    while _o < _w:
        _c = min(CHUNK, _w - _o)
        CHUNKS.append((_ti, _o, _c))
        _o += _c
NT = len(CHUNKS)
LOG_CLAMP = -100.0

# Optional instrumentation knobs for a driver script (harness never sets them).
TRACE = False
LAST_RESULTS = None

_NC_CACHE = None


def _build():
    f32 = mybir.dt.float32
    i32 = mybir.dt.int32
    Ln = mybir.ActivationFunctionType.Ln
    Copy = mybir.ActivationFunctionType.Copy

    nc = bacc.Bacc("TRN2")
    x_in = nc.declare_dram_parameter("x", [P, FD], f32, isOutput=False)
    g_in = nc.declare_dram_parameter("gt", [P, FD], i32, isOutput=False)
    # packed output: column groups [A | B | C | S_act | S_dve], NT each
    out_all = nc.declare_dram_parameter("out_all", [P, 5 * NT], f32, isOutput=True)

    with tile.TileContext(nc) as tc, ExitStack() as ctx:
        xp = ctx.enter_context(tc.tile_pool(name="xp", bufs=3))
        gp = ctx.enter_context(tc.tile_pool(name="gp", bufs=3))
        lp = ctx.enter_context(tc.tile_pool(name="lp", bufs=3))
        jp = ctx.enter_context(tc.tile_pool(name="jp", bufs=1))
        accp = ctx.enter_context(tc.tile_pool(name="accp", bufs=1))

        acc = accp.tile([P, 5 * NT], f32)

        def col(group, i):
            j = group * NT + i
            return acc[:, j : j + 1]

        # issue all input DMAs in consumption order on the SP ring; tile
        # pool recycling (bufs=3) paces the ring
        xts, gts = {}, {}
        dma_plan = []
        off = 0
        for ti, w in enumerate(DMA_TILES):
            dma_plan.append((ti, off, w))
            off += w

        next_dma = 0

        def issue_dma():
            nonlocal next_dma
            ti, o, w = dma_plan[next_dma]
            next_dma += 1
            xt = xp.tile([P, w], f32, tag="xt")
            gt_t = gp.tile([P, w], i32, tag="gt")
            nc.sync.dma_start(xt[:], x_in[:, o : o + w])
            nc.sync.dma_start(gt_t[:], g_in[:, o : o + w])
            xts[ti], gts[ti] = xt, gt_t

        # prime the pipeline with all transfers the pools allow; the tile
        # framework's semaphores handle the rest of the pacing
        for i, (ti, o, w) in enumerate(CHUNKS):
            while next_dma < len(dma_plan) and dma_plan[next_dma][0] <= ti:
                issue_dma()
            xt, gt_t = xts[ti], gts[ti]
            xc = xt[:, o : o + w]
            gc = gt_t[:, o : o + w]

            lnx = lp.tile([P, w], f32, tag="lnx")
            ln1 = lp.tile([P, w], f32, tag="ln1")
            nc.scalar.activation(lnx[:], xc, Ln)
            nc.scalar.activation(
                ln1[:], xc, Ln, bias=1.0, scale=-1.0,
                accum_out=col(2, i),
            )

            ha = (w * 75) // 128  # ~0.59 of S columns on ACT, rest DVE
            junk3 = jp.tile([P, w], f32, tag="junk3")
            # S (DVE part): needs only gt - before A/B to fill DVE's bubble
            nc.vector.scalar_tensor_tensor(
                junk3[:, ha:], gc[:, ha:], 0.0, gc[:, ha:],
                AluOpType.mult, AluOpType.add,
                accum_out=col(4, i),
            )

            junk = jp.tile([P, w], f32, tag="junk")
            nc.vector.scalar_tensor_tensor(
                junk[:], lnx[:], LOG_CLAMP, gc,
                AluOpType.max, AluOpType.mult,
                accum_out=col(0, i),
            )
            junk2 = jp.tile([P, w], f32, tag="junk")
            nc.vector.scalar_tensor_tensor(
                junk2[:], ln1[:], LOG_CLAMP, gc,
                AluOpType.max, AluOpType.mult,
                accum_out=col(1, i),
            )

            # S (ACT part)
            nc.scalar.activation(
                junk3[:, :ha], gc[:, :ha], Copy, accum_out=col(3, i)
            )

        nc.sync.dma_start(out_all[:], acc[:])

    nc.compile()
    return nc


def get_nc():
    global _NC_CACHE
    if _NC_CACHE is None:
        _NC_CACHE = _build()
    return _NC_CACHE


def make_in_maps(x, gt):
    x = np.ascontiguousarray(np.asarray(x, dtype=np.float32).reshape(-1))
    gt = np.ascontiguousarray(np.asarray(gt, dtype=np.int32).reshape(-1))
    assert x.shape == (N_TOTAL,) and gt.shape == (N_TOTAL,)
    in_maps = []
    for c in range(N_CORES):
        sl = slice(c * PER_CORE, (c + 1) * PER_CORE)
        in_maps.append({
            "x": x[sl].reshape(P, FD),
            "gt": gt[sl].reshape(P, FD),
        })
    return in_maps


def combine(results):
    """All-reduce the per-core partial sums and finish the loss formula."""
    A = B = C = S = 0.0
    for r in results:
        o = r["out_all"].astype(np.float64)
        A += o[:, 0 * NT : 1 * NT].sum()
        B += o[:, 1 * NT : 2 * NT].sum()
        C += o[:, 2 * NT : 3 * NT].sum()
        S += o[:, 3 * NT : 5 * NT].sum()   # S_act + S_dve
    n = float(N_TOTAL)
    result = -(A / (2.0 * S) + (C - B) / (2.0 * (n - S)))
    return np.array(result, dtype=np.float32)


def kernel(x, gt):
    global LAST_RESULTS
    nc = get_nc()
    in_maps = make_in_maps(x, gt)
    br = run_bass_kernel_spmd(nc, in_maps, list(range(N_CORES)))
    LAST_RESULTS = br
    return combine(br.results)
